# revision 9
# baseline (speedup 1.0000x reference)
"""Trainium2 Bass kernel for nn_FFT_TREND (B=32, N=256, T=2048, K=5).

Pure data-parallel over 8 NeuronCores: each core handles 4 samples.

Per-core pipeline (all on device):
  A. Load x, even/odd fold along t, PE-transpose to [t, ch] layout.
  B. Real DFT via fp32 matmuls (folded: cos on xe, sin on xo), |X| magnitude,
     channel-mean via Sqrt+accum, top-5 bins via max/max_index.
  C. Per (sample, kernel-size): moving average via extended cumsum array G
     (affine tails encode replicate padding), data-dependent shifts done as
     dynamic-slice reads with register offsets looked up from host-built
     tables; branchless rank-1 correction handles kernel sizes > 2T.

All inputs (x + every constant table, int tables bitcast to f32) are packed
into ONE [128, CBLOB] dram tensor per core: the axon dispatch path costs
~0.2ms per buffer handle per call, so 2 handles (blob + out) beat 13.
"""
import sys
sys.path.insert(0, "/opt/trn_rl_repo")
import os
import numpy as np

import concourse.bacc as bacc
import concourse.mybir as mybir
from concourse.bass import ds
from concourse.expressions import smin
from concourse.bass_utils import run_bass_kernel_spmd
from concourse.tile import TileContext

P = 128
B, N, T, KTOP = 32, 256, 2048, 5
FS = 120.0
NCORES = 8
BL = B // NCORES          # 4 samples per core
NBLK = N // P             # 2 channel blocks
NBINS = 1024              # bins 1..1024 (DC killed)
KC_E = 9                  # xe t-chunks (t = 0..1151, data 0..1024)
KC_O = 8                  # xo t-chunks (t = 0..1023)
KCHT = KC_E + KC_O        # 17
UW = KCHT * P             # 2176 cols per (sample, block) unit in xT
GW = 3 * T + 1            # 6145 cols in extended-cumsum array G
dt = mybir.dt

# ---- packed-blob column offsets (one [128, CBLOB] f32 tensor per core) ----
OFF_WC = 0                      # 36 x [128,256] cos-DFT chunks
OFF_WS = OFF_WC + KC_E * 4 * 256    # 32 x [128,256] sin-DFT chunks
OFF_RAMP = OFF_WS + KC_O * 4 * 256  # [128,2048]
OFF_IDENT = OFF_RAMP + T            # [128,128]
OFF_INVK = OFF_IDENT + P            # [128,1024]
OFF_DL = OFF_INVK + NBINS           # [128,16]
OFF_DH = OFF_DL + 16                # [128,16]
OFF_ITAB = OFF_DH + 16              # row 0: hi|lo|last int32 bitcast, 3x1024
OFF_X = OFF_ITAB + 3 * NBINS        # 8 x [128,2048] sample-block units
CBLOB = OFF_X + 2 * BL * T

_cache = {}


def _host_consts():
    """Builds the constant part of the blob: [128, OFF_X] f32."""
    if "cblob" in _cache:
        return _cache["cblob"]
    cb = np.zeros((P, OFF_X), dtype=np.float32)
    pos = np.arange(NBINS)
    idxf = (pos + 1).astype(np.float32)
    freq = idxf * np.float32(FS / T)            # exact fp32 (FS/T = 15/256)
    k = np.floor(np.float32(T) / freq).astype(np.int64)  # replicates reference
    p = (k - 1) // 2
    q = k - p                      # hi shift: p+1 odd k, p+2 even k
    pc = np.minimum(p, T - 1)
    qc = np.minimum(q, T)
    dl = (p - pc).astype(np.float64)
    dh = (q - qc).astype(np.float64)
    invk = (1.0 / k.astype(np.float32)).astype(np.float32)
    hi = (2048 + qc).astype(np.int32)
    lo = (2048 - pc).astype(np.int32)
    last = np.where(k % 2 == 0, 2046, 2047).astype(np.int32)
    cb[0, OFF_ITAB:OFF_ITAB + NBINS] = hi.view(np.float32)
    cb[0, OFF_ITAB + NBINS:OFF_ITAB + 2 * NBINS] = lo.view(np.float32)
    cb[0, OFF_ITAB + 2 * NBINS:OFF_ITAB + 3 * NBINS] = last.view(np.float32)
    cb[:, OFF_INVK:OFF_INVK + NBINS] = invk[None, :]
    cb[:, OFF_DL:OFF_DL + 16] = (dl / k).astype(np.float32)[None, :16]
    cb[:, OFF_DH:OFF_DH + 16] = (dh / k).astype(np.float32)[None, :16]
    cb[:, OFF_RAMP:OFF_RAMP + T] = np.arange(1, T + 1, dtype=np.float32)[None, :]
    cb[:, OFF_IDENT:OFF_IDENT + P] = np.eye(P, dtype=np.float32)
    # DFT matrices (folded real DFT, bins 1..1024)
    tt = np.arange(KC_E * P, dtype=np.float64)          # 0..1151
    bins = np.arange(1, NBINS + 1, dtype=np.float64)
    ang = 2.0 * np.pi / T * tt[:, None] * bins[None, :]
    wc = np.cos(ang)
    wc[tt > 1024, :] = 0.0
    ws = np.sin(ang[:KC_O * P])                          # t = 0..1023
    # layout [kc, g, 128, 256] with col = fi*128 + j, fc = 2g+fi, bin = fc*128+j+1
    wc4 = (wc.reshape(KC_E, P, 4, 2, P).transpose(0, 2, 1, 3, 4)
           .reshape(KC_E * 4, P, 2 * P)).astype(np.float32)
    ws4 = (ws.reshape(KC_O, P, 4, 2, P).transpose(0, 2, 1, 3, 4)
           .reshape(KC_O * 4, P, 2 * P)).astype(np.float32)
    cb[:, OFF_WC:OFF_WC + KC_E * 4 * 256] = (
        wc4.transpose(1, 0, 2).reshape(P, KC_E * 4 * 256))
    cb[:, OFF_WS:OFF_WS + KC_O * 4 * 256] = (
        ws4.transpose(1, 0, 2).reshape(P, KC_O * 4 * 256))
    _cache["cblob"] = cb
    return cb


def _build():
    if "nc" in _cache:
        return _cache["nc"]
    nc = bacc.Bacc("TRN2", target_bir_lowering=False, debug=False,
                   enable_partition_id=False)
    GEARLY = os.environ.get("KERNEL_GEARLY", "0") == "1"
    ACT_SCALE = os.environ.get("KERNEL_ACT_SCALE", "0") == "1"
    POOL_PREP = os.environ.get("KERNEL_POOL_PREP", "0") == "1"
    DVE = [mybir.EngineType.DVE]
    A = mybir.AluOpType
    AF = mybir.ActivationFunctionType

    cf = nc.dram_tensor("cf", (P, CBLOB), dt.float32, kind="ExternalInput").ap()
    out_t = nc.dram_tensor("out", (BL, N, KTOP, T), dt.float32, kind="ExternalOutput").ap()

    def x_sl(u):
        return cf[:, OFF_X + u * T:OFF_X + (u + 1) * T]

    with TileContext(nc) as tc:
        with tc.tile_pool(name="const", bufs=1) as cpool, \
             tc.tile_pool(name="xT", bufs=1) as xTpool, \
             tc.tile_pool(name="small", bufs=1) as spool:
            identt = cpool.tile([P, P], dt.float32)
            nc.sync.dma_start(identt, cf[:, OFF_IDENT:OFF_IDENT + P])
            rampt = cpool.tile([P, T], dt.float32)
            nc.sync.dma_start(rampt, cf[:, OFF_RAMP:OFF_RAMP + T])
            hit = cpool.tile([1, NBINS], dt.int32)
            nc.sync.dma_start(
                hit, cf[0:1, OFF_ITAB:OFF_ITAB + NBINS].bitcast(dt.int32))
            lot = cpool.tile([1, NBINS], dt.int32)
            nc.sync.dma_start(
                lot, cf[0:1, OFF_ITAB + NBINS:OFF_ITAB + 2 * NBINS].bitcast(dt.int32))
            lastt = cpool.tile([1, NBINS], dt.int32)
            nc.sync.dma_start(
                lastt, cf[0:1, OFF_ITAB + 2 * NBINS:OFF_ITAB + 3 * NBINS].bitcast(dt.int32))
            invkt = cpool.tile([P, NBINS], dt.float32)
            nc.sync.dma_start(invkt, cf[:, OFF_INVK:OFF_INVK + NBINS])
            dlinvkt = cpool.tile([P, 16], dt.float32)
            nc.sync.dma_start(dlinvkt, cf[:, OFF_DL:OFF_DL + 16])
            dhinvkt = cpool.tile([P, 16], dt.float32)
            nc.sync.dma_start(dhinvkt, cf[:, OFF_DH:OFF_DH + 16])

            xTt = xTpool.tile([P, 2 * BL * UW], dt.float32)
            xTr = xTt[:].rearrange("p (u c) -> p u c", c=UW)

            # ---------------- Phase A: fold + transpose ----------------
            with tc.tile_pool(name="xnat", bufs=2) as xnp, \
                 tc.tile_pool(name="fold", bufs=2) as fp, \
                 tc.tile_pool(name="tpps", bufs=2, space="PSUM") as tpp:
                for b in range(BL):
                    for blk in range(NBLK):
                        u = b * NBLK + blk
                        xn = xnp.tile([P, T], dt.float32, tag="xn")
                        nc.sync.dma_start(xn, x_sl(u))
                        xe = fp.tile([P, KC_E * P], dt.float32, tag="xe")
                        xo = fp.tile([P, KC_O * P], dt.float32, tag="xo")
                        nc.vector.tensor_tensor(
                            xe[:, 1:1024], xn[:, 1:1024], xn[:, 2047:1024:-1], A.add)
                        nc.vector.tensor_copy(xe[:, 0:1], xn[:, 0:1])
                        nc.vector.tensor_copy(xe[:, 1024:1025], xn[:, 1024:1025])
                        nc.vector.memset(xe[:, 1025:KC_E * P], 0.0)
                        nc.vector.tensor_tensor(
                            xo[:, 1:1024], xn[:, 1:1024], xn[:, 2047:1024:-1], A.subtract)
                        nc.vector.memset(xo[:, 0:1], 0.0)
                        for grp in range(5):
                            c0 = grp * 4
                            ncks = min(4, KCHT - c0)
                            tp = tpp.tile([P, 512], dt.float32, tag="tp")
                            for ci in range(ncks):
                                c = c0 + ci
                                src = (xe[:, c * P:(c + 1) * P] if c < KC_E
                                       else xo[:, (c - KC_E) * P:(c - KC_E + 1) * P])
                                nc.tensor.transpose(
                                    tp[:, ci * P:(ci + 1) * P], src, identt)
                            nc.scalar.activation(
                                xTt[:, u * UW + c0 * P: u * UW + c0 * P + ncks * P],
                                tp[:, 0:ncks * P], AF.Copy)

            # ---------------- Phases B+C interleaved ----------------
            # DFT runs in two 2-sample passes; each pass's moving-average work
            # is emitted immediately after it so its DVE/DMA overlaps the next
            # pass's matmuls instead of queuing behind them.
            idxrows = []
            with tc.tile_pool(name="wdma", bufs=int(os.environ.get("BUF_W", "3"))) as wp, \
                 tc.tile_pool(name="dftps", bufs=1, space="PSUM") as dpp, \
                 tc.tile_pool(name="mtps", bufs=1, space="PSUM") as mtp, \
                 tc.tile_pool(name="sq", bufs=int(os.environ.get("BUF_SQ", "2"))) as sqp, \
                 tc.tile_pool(name="xnat2", bufs=int(os.environ.get("BUF_XN", "2"))) as xnp2, \
                 tc.tile_pool(name="Gp", bufs=2) as gp, \
                 tc.tile_pool(name="colp", bufs=2) as clp, \
                 tc.tile_pool(name="magp", bufs=2) as mgp, \
                 tc.tile_pool(name="comb", bufs=int(os.environ.get("BUF_COMB", "2"))) as cbp:

                _plan = os.environ.get("KERNEL_PLAN", "2,2")
                PASSES = []           # (first_sample, n_samples)
                _s = 0
                for _n in [int(v) for v in _plan.split(",")]:
                    PASSES.append((_s, _n))
                    _s += _n
                assert _s == BL

                def emit_dft_half(half):
                    b0, SP = PASSES[half]
                    u0 = b0 * 2
                    magsum = mgp.tile([P, 8 * SP], dt.float32, tag="magsum", name="magsum")
                    for g in range(4):
                        psC = []
                        psS = []
                        for i in range(2):
                            psc_i = dpp.tile([P, 256 * SP], dt.float32, tag=f"psc{i}")
                            pss_i = dpp.tile([P, 256 * SP], dt.float32, tag=f"pss{i}")
                            psC.append(psc_i)
                            psS.append(pss_i)
                        for kc in range(KC_E):
                            wct = wp.tile([P, 2 * P], dt.float32, tag="wc")
                            nc.sync.dma_start(
                                wct,
                                cf[:, OFF_WC + (kc * 4 + g) * 256:
                                   OFF_WC + (kc * 4 + g) * 256 + 256])
                            wst = None
                            if kc < KC_O:
                                wst = wp.tile([P, 2 * P], dt.float32, tag="ws")
                                nc.sync.dma_start(
                                    wst,
                                    cf[:, OFF_WS + (kc * 4 + g) * 256:
                                       OFF_WS + (kc * 4 + g) * 256 + 256])
                            for fi in range(2):
                                # fp32 matmuls: fp32r/bf16 would flip top-5
                                # picks (min 5th/6th mag gap on the graded
                                # input is 4.1e-5, under single-pass PE noise).
                                rhs_e = xTr[:, u0:u0 + 2 * SP, kc * P:(kc + 1) * P]
                                nc.tensor.matmul(
                                    psC[fi], wct[:, fi * P:(fi + 1) * P], rhs_e,
                                    start=(kc == 0), stop=(kc == KC_E - 1),
                                    skip_group_check=True)
                                if kc < KC_O:
                                    rhs_o = xTr[:, u0:u0 + 2 * SP,
                                                (KC_E + kc) * P:(KC_E + kc + 1) * P]
                                    nc.tensor.matmul(
                                        psS[fi], wst[:, fi * P:(fi + 1) * P], rhs_o,
                                        start=(kc == 0), stop=(kc == KC_O - 1),
                                        skip_group_check=True)
                        for fi in range(2):
                            fc = 2 * g + fi
                            sqc = sqp.tile([P, 256 * SP], dt.float32, tag="sqc")
                            sqs = sqp.tile([P, 256 * SP], dt.float32, tag="sqs")
                            scr = sqp.tile([P, 256], dt.float32, tag="scr")
                            nc.scalar.activation(sqc, psC[fi], AF.Square)
                            nc.scalar.activation(sqs, psS[fi], AF.Square)
                            nc.vector.tensor_tensor(sqc, sqc, sqs, A.add)
                            for bh in range(SP):
                                nc.scalar.activation(
                                    scr, sqc[:, bh * 256:(bh + 1) * 256], AF.Sqrt,
                                    accum_out=magsum[:, fc * SP + bh: fc * SP + bh + 1])
                    mag_h = mgp.tile([SP, NBINS], dt.float32, tag="mag_h", name="mag_h")
                    mt = mtp.tile([8 * SP, P], dt.float32, tag="mt", name="mt")
                    nc.tensor.transpose(mt, magsum[:, 0:8 * SP], identt)
                    mtsb = mgp.tile([8 * SP, P], dt.float32, tag="mtsb", name="mtsb")
                    nc.scalar.activation(mtsb, mt, AF.Copy)
                    for fc in range(8):
                        nc.sync.dma_start(
                            mag_h[0:SP, fc * P:(fc + 1) * P],
                            mtsb[fc * SP:fc * SP + SP, :])
                    mx = mgp.tile([SP, 8], dt.float32, tag="mx", name="mx")
                    mi = mgp.tile([SP, 8], dt.uint32, tag="mi", name="mi")
                    nc.vector.max(out=mx, in_=mag_h)
                    nc.vector.max_index(mi, mx, mag_h)
                    idxrow = mgp.tile([1, 8 * SP], dt.uint32, tag="idxrow", name="idxrow")
                    nc.sync.dma_start(idxrow, mi)
                    idxrows.append(idxrow)

                def emit_sample_G(b):
                    """Index-independent part: cumsum array G + edge columns.
                    Emitted before the DFT half so the DVE scans/ACT ramps run
                    under the matmuls instead of serializing after them."""
                    Gs, cols = [], []
                    for blk in range(NBLK):
                        xn = xnp2.tile([P, T], dt.float32, tag="xn2", name="xn2")
                        nc.sync.dma_start(xn, x_sl(b * NBLK + blk))
                        G = gp.tile([P, GW], dt.float32, tag="G", name="G")
                        cl = clp.tile([P, 8], dt.float32, tag=f"cols{blk}",
                                      name=f"cols{blk}")
                        nc.vector.tensor_copy(cl[:, 0:1], xn[:, 0:1])
                        nc.vector.tensor_copy(cl[:, 1:2], xn[:, 2047:2048])
                        nc.vector.tensor_scalar_mul(cl[:, 2:3], cl[:, 0:1], -2049.0)
                        nc.vector.tensor_tensor_scan(
                            G[:, T + 1:2 * T + 1], xn, xn, 0.0, A.add, A.bypass)
                        nc.vector.memset(G[:, T:T + 1], 0.0)
                        nc.scalar.activation(
                            G[:, 0:T], rampt, AF.Identity,
                            bias=cl[:, 2:3], scale=cl[:, 0:1])
                        nc.scalar.activation(
                            G[:, 2 * T + 1:GW], rampt, AF.Identity,
                            bias=G[:, 2 * T:2 * T + 1], scale=cl[:, 1:2])
                        Gs.append(G)
                        cols.append(cl)
                    return Gs, cols

                def emit_sample_comb(b, Gs, cols, spread=False):
                    """Index-dependent part. Engine split per (kk, blk):
                    window subtract + last-col fix on DVE, per-kk scalar prep
                    on Pool, final scale+bias on ACT — three engines pipeline
                    instead of everything queuing on DVE."""
                    _half = max(h for h, (s0, _) in enumerate(PASSES) if s0 <= b)
                    _boff = b - PASSES[_half][0]
                    LD = list(DVE)
                    if spread and ACT_SCALE:
                        LD.append(mybir.EngineType.Activation)
                    if spread and POOL_PREP:
                        LD.append(mybir.EngineType.Pool)
                    for kk in range(KTOP):
                        j = _boff * 8 + kk
                        idx = nc.values_load(
                            idxrows[_half][0:1, j:j + 1], engines=LD,
                            min_val=0, max_val=NBINS - 1,
                            skip_runtime_bounds_check=True)
                        hi_s = nc.values_load(
                            hit[0:1, ds(idx, 1)], engines=DVE,
                            min_val=2065, max_val=4096,
                            skip_runtime_bounds_check=True)
                        lo_s = nc.values_load(
                            lot[0:1, ds(idx, 1)], engines=DVE,
                            min_val=1, max_val=2032,
                            skip_runtime_bounds_check=True)
                        last = nc.values_load(
                            lastt[0:1, ds(idx, 1)], engines=DVE,
                            min_val=2046, max_val=2047,
                            skip_runtime_bounds_check=True)
                        for blk in range(NBLK):
                            G, cl = Gs[blk], cols[blk]
                            comb = cbp.tile([P, T], dt.float32, tag="comb",
                                            name="comb")
                            nc.vector.tensor_tensor(
                                comb, G[:, ds(hi_s, T)], G[:, ds(lo_s, T)],
                                A.subtract)
                            nc.vector.tensor_tensor(
                                comb[:, T - 1:T],
                                G[:, ds(hi_s + last, 1)], G[:, ds(lo_s + last, 1)],
                                A.subtract)
                            idxc = smin(idx, 15)   # dl=dh=0 for idx >= 9
                            _prep = nc.gpsimd if (spread and POOL_PREP) else nc.vector
                            _prep.tensor_scalar_mul(
                                cl[:, 4:5], cl[:, 0:1], dlinvkt[:, ds(idxc, 1)])
                            _prep.scalar_tensor_tensor(
                                cl[:, 5:6], cl[:, 1:2], dhinvkt[:, ds(idxc, 1)],
                                cl[:, 4:5], A.mult, A.add)
                            if spread and ACT_SCALE:
                                nc.scalar.activation(
                                    comb, comb, AF.Identity,
                                    scale=invkt[:, ds(idx, 1)], bias=cl[:, 5:6])
                            else:
                                nc.vector.tensor_scalar(
                                    comb, comb, invkt[:, ds(idx, 1)], cl[:, 5:6],
                                    A.mult, A.add)
                            nc.sync.dma_start(
                                out_t[b, blk * P:(blk + 1) * P, kk, :], comb)

                for half, (s0, sp_n) in enumerate(PASSES):
                    last_half = half == len(PASSES) - 1
                    if GEARLY:
                        Gc0 = emit_sample_G(s0)
                        emit_dft_half(half)
                        emit_sample_comb(s0, *Gc0, spread=last_half)
                        for bh in range(1, sp_n):
                            Gc = emit_sample_G(s0 + bh)
                            emit_sample_comb(s0 + bh, *Gc, spread=last_half)
                    else:
                        emit_dft_half(half)
                        for bh in range(sp_n):
                            Gc = emit_sample_G(s0 + bh)
                            emit_sample_comb(s0 + bh, *Gc, spread=last_half)

    nc.compile()
    _cache["nc"] = nc
    return nc


def _in_maps(x):
    cb = _host_consts()
    x = np.ascontiguousarray(x, dtype=np.float32)
    maps = []
    for c in range(NCORES):
        blob = np.empty((P, CBLOB), dtype=np.float32)
        blob[:, :OFF_X] = cb
        xc = x[c * BL:(c + 1) * BL]                     # (BL, N, T)
        blob[:, OFF_X:] = (xc.reshape(BL, NBLK, P, T)
                           .transpose(2, 0, 1, 3).reshape(P, 2 * BL * T))
        maps.append(dict(cf=blob))
    return maps


def _run(x, **kw):
    nc = _build()
    return run_bass_kernel_spmd(nc, _in_maps(x), core_ids=list(range(NCORES)), **kw)


def _get_exec():
    """Cached PJRT executable over the 8 axon cores (mirrors
    bass2jax.run_bass_via_pjrt's multi-core branch, but jit-cached)."""
    if "exec" in _cache:
        return _cache["exec"]
    import jax
    from jax.sharding import Mesh, PartitionSpec
    from jax.experimental.shard_map import shard_map
    import concourse.bass2jax as b2j
    import concourse.mybir as mybir_

    b2j.install_neuronx_cc_hook()
    nc = _build()
    pname = nc.partition_id_tensor.name if nc.partition_id_tensor else None
    in_names, out_names, out_avals, zero_shapes = [], [], [], []
    for alloc in nc.m.functions[0].allocations:
        if not isinstance(alloc, mybir_.MemoryLocationSet):
            continue
        name = alloc.memorylocations[0].name
        if alloc.kind == "ExternalInput":
            if name != pname:
                in_names.append(name)
        elif alloc.kind == "ExternalOutput":
            shape = tuple(alloc.tensor_shape)
            np_dt = mybir_.dt.np(alloc.dtype)
            out_names.append(name)
            out_avals.append(jax.core.ShapedArray(shape, np_dt))
            zero_shapes.append((shape, np_dt))
    n_params = len(in_names)
    all_in_names = in_names + out_names
    if pname is not None:
        all_in_names = all_in_names + [pname]

    def _body(*args):
        operands = list(args)
        if pname is not None:
            operands.append(b2j.partition_id_tensor())
        outs = b2j._bass_exec_p.bind(
            *operands,
            out_avals=tuple(out_avals),
            in_names=tuple(all_in_names),
            out_names=tuple(out_names),
            lowering_input_output_aliases=(),
            sim_require_finite=True,
            sim_require_nnan=True,
            nc=nc,
        )
        return tuple(outs)

    devices = jax.devices()[:NCORES]
    mesh = Mesh(np.asarray(devices), ("core",))
    nio = n_params + len(out_names)
    sharded = jax.jit(
        shard_map(_body, mesh=mesh,
                  in_specs=(PartitionSpec("core"),) * nio,
                  out_specs=(PartitionSpec("core"),) * len(out_names),
                  check_rep=False),
        donate_argnums=tuple(range(n_params, nio)),
        keep_unused=True,
    )
    ex = dict(sharded=sharded, in_names=in_names, out_names=out_names,
              out_avals=out_avals, zero_shapes=zero_shapes, mesh=mesh)
    _cache["exec"] = ex
    return ex


def _concat_inputs(x):
    ex = _get_exec()
    maps = _in_maps(x)
    return [np.concatenate([maps[c][n] for c in range(NCORES)], axis=0)
            for n in ex["in_names"]]


def _make_zeros(on_device=False):
    ex = _get_exec()
    if on_device:
        import jax.numpy as jnp
        from jax.sharding import NamedSharding, PartitionSpec
        sh = NamedSharding(ex["mesh"], PartitionSpec("core"))
        return [jnp.zeros((NCORES * s[0], *s[1:]), d, device=sh)
                for s, d in ex["zero_shapes"]]
    return [np.zeros((NCORES * s[0], *s[1:]), d) for s, d in ex["zero_shapes"]]


def kernel(x):
    try:
        ex = _get_exec()
        outs = ex["sharded"](*_concat_inputs(x), *_make_zeros())
        out = np.asarray(outs[ex["out_names"].index("out")])
        return out.reshape(NCORES, BL, N, KTOP, T).reshape(B, N, KTOP, T)
    except Exception:
        res = _run(x)
        return np.concatenate([res.results[c]["out"] for c in range(NCORES)],
                              axis=0)
